# revision 13
# baseline (speedup 1.0000x reference)
"""MoE grouped-GEMM expert FFN (SwiGLU) on 8 Trainium2 NeuronCores.

Expert-parallel sharding: tokens arrive pre-grouped by expert with uniform
group size g = T/E = 1024, so core c owns experts [4c, 4c+4) and token rows
[c*4096, (c+1)*4096). No cross-core communication is needed.

Per-core schedule (per expert e):
    phase1: gu^T[m] = sum_k w13[e,k,m]^T @ xt[k]      (PE, 4 PSUM banks/m)
            h^T[m]  = silu(gate^T) * up^T             (ACT silu + DVE mul)
    phase2: out[mt] = sum_k h^T[k,mt]^T @ w2[e,k]     (PE, 4 PSUM banks/mt)
            bf16 copy to SBUF (DVE) -> DMA store      (ACT HWDGE ring)

Perf structure (steady-state device time ~690us/core vs 656us bf16 PE
roofline):
  - One rotating PSUM tag (8 banks, 4 banks/group -> 2-deep pipelining) so
    the PE never waits on the SwiGLU / output-copy epilogues.
  - Weights & activations stream with deep prefetch: per-expert batched
    DMAs issued in need-order on the SP ring; output stores ride the ACT
    ring so they can't head-of-line-block the prefetches.
  - DVE multiplies silu(gate) (SBUF) by up (PSUM directly) - no ACT copy.
  - Output is written bf16 (host upcasts) halving store traffic.
"""

import sys

if "/opt/trn_rl_repo" not in sys.path:
    sys.path.insert(0, "/opt/trn_rl_repo")

import ml_dtypes
import numpy as np

import concourse.bacc as bacc
import concourse.mybir as mybir
from concourse import tile
from concourse.bass_utils import run_bass_kernel_spmd

BF16 = mybir.dt.bfloat16
F32 = mybir.dt.float32
NPBF16 = ml_dtypes.bfloat16

N_CORES = 8
E = 32
H = 2048
I = 1024
T = 32768
EPC = E // N_CORES          # experts per core = 4
G = T // E                  # tokens per expert = 1024
ROWS = EPC * G              # token rows per core = 4096
KH = H // 128               # 16 contraction tiles for GEMM1
KI = I // 128               # 8 contraction tiles for GEMM2 / m-tiles


def build_nc(nrep=1):
    nc = bacc.Bacc()
    # xt:  x^T per expert, [e][k][128 h][1024 tok]
    xt_d = nc.declare_dram_parameter("xt", [EPC, KH, 128, G], BF16, isOutput=False)
    # w13: [e][m][128 j][k][s: gate|up][128 col] -> per (e,m) one contiguous 1MB
    w13_d = nc.declare_dram_parameter("w13", [EPC, KI, 128, KH, 2, 128], BF16, isOutput=False)
    # w2:  [e][k][128 i][2048 h]
    w2_d = nc.declare_dram_parameter("w2", [EPC, KI, 128, H], BF16, isOutput=False)
    out_d = nc.declare_dram_parameter("out", [ROWS, H], BF16, isOutput=True)

    with tile.TileContext(nc) as tc:
        with (
            tc.tile_pool(name="xt", bufs=2) as xt_pool,
            tc.tile_pool(name="w13", bufs=4) as w13_pool,
            tc.tile_pool(name="w2", bufs=10) as w2_pool,
            tc.tile_pool(name="h", bufs=2) as h_pool,
            tc.tile_pool(name="tmp", bufs=4) as tmp_pool,
            tc.tile_pool(name="ot", bufs=3) as ot_pool,
            tc.tile_pool(name="ps", bufs=8, space="PSUM") as ps_pool,
        ):
            from contextlib import nullcontext

            loop_ctx = tc.For_i(0, nrep, 1) if nrep > 1 else nullcontext()
            with loop_ctx:
                def load_w13_m(e, m):
                    t = w13_pool.tile([128, KH, 2, 128], BF16, tag="w13",
                                      name=f"w13_{e}_{m}")
                    nc.sync.dma_start(t[:], w13_d[e, m])
                    return t

                def load_expert_head(e):
                    """xt + the first w13 m-block - just enough for phase1(e, m=0).

                    For e=0 (the cold start: nothing else hides these loads)
                    xt streams per k-tile, interleaved with w13 m0, so the
                    first matmul can issue after ~1.3MB instead of ~5.2MB."""
                    xt_sb = xt_pool.tile([128, KH, G], BF16, tag="xt", name=f"xt_{e}")
                    if e == 0:
                        nc.sync.dma_start(xt_sb[:, 0, :], xt_d[e, 0])
                        w13_head = [load_w13_m(e, 0)]
                        for k in range(1, KH):
                            nc.sync.dma_start(xt_sb[:, k, :], xt_d[e, k])
                    else:
                        nc.sync.dma_start(xt_sb[:], xt_d[e].transpose([1, 0, 2]))
                        w13_head = [load_w13_m(e, 0)]
                    return xt_sb, w13_head

                def load_w2(e):
                    w2_sb = []
                    for k in range(KI):
                        t = w2_pool.tile([128, H], BF16, tag="w2", name=f"w2_{e}_{k}")
                        nc.sync.dma_start(t[:], w2_d[e, k])
                        w2_sb.append(t)
                    return w2_sb

                def phase1_m(e, m, xt_sb, w13_sb, h_sb):
                    pg = [ps_pool.tile([128, 512], F32, tag="ps", name=f"pg{n}_{e}_{m}")
                          for n in range(2)]
                    pu = [ps_pool.tile([128, 512], F32, tag="ps", name=f"pu{n}_{e}_{m}")
                          for n in range(2)]
                    for k in range(KH):
                        wg = w13_sb[m][:, k, 0, :]
                        wu = w13_sb[m][:, k, 1, :]
                        for n in range(2):
                            nc.tensor.matmul(
                                pg[n][:], wg, xt_sb[:, k, n * 512:(n + 1) * 512],
                                start=(k == 0), stop=(k == KH - 1),
                            )
                        for n in range(2):
                            nc.tensor.matmul(
                                pu[n][:], wu, xt_sb[:, k, n * 512:(n + 1) * 512],
                                start=(k == 0), stop=(k == KH - 1),
                            )
                    for n in range(2):
                        tmp = tmp_pool.tile([128, 512], F32, tag="tmp",
                                            name=f"tmp_{e}_{m}_{n}")
                        nc.scalar.activation(
                            tmp[:], pg[n][:], mybir.ActivationFunctionType.Silu
                        )
                        nc.vector.tensor_mul(
                            h_sb[:, m, n * 512:(n + 1) * 512], tmp[:], pu[n][:]
                        )

                def phase2(e, h_sb, w2_sb):
                    for mt in range(KI):
                        po = [ps_pool.tile([128, 512], F32, tag="ps",
                                           name=f"po{n}_{e}_{mt}")
                              for n in range(4)]
                        for k in range(KI):
                            hk = h_sb[:, k, mt * 128:(mt + 1) * 128]
                            for n in range(4):
                                nc.tensor.matmul(
                                    po[n][:], hk, w2_sb[k][:, n * 512:(n + 1) * 512],
                                    start=(k == 0), stop=(k == KI - 1),
                                )
                        ot = ot_pool.tile([128, H], BF16, tag="ot", name=f"ot_{e}_{mt}")
                        for n in range(4):
                            nc.vector.tensor_copy(ot[:, n * 512:(n + 1) * 512], po[n][:])
                        rows = slice(e * G + mt * 128, e * G + (mt + 1) * 128)
                        nc.scalar.dma_start(out_d[rows, :], ot[:])

                # plain expert schedule; DMA issue order per expert matches
                # need order: xt(e), w13(e, m0..7), w2(e, k0..7). (An
                # interleaved variant running phase1(e+1, m=0) before
                # phase2(e) measured 124us/iter SLOWER - do not revive it.)
                for e in range(EPC):
                    xt_sb, w13_sb = load_expert_head(e)
                    for m in range(1, KI):
                        w13_sb.append(load_w13_m(e, m))
                    w2_sb = load_w2(e)
                    h_sb = h_pool.tile([128, KI, G], BF16, tag="h", name=f"h_{e}")
                    for m in range(KI):
                        phase1_m(e, m, xt_sb, w13_sb, h_sb)
                    phase2(e, h_sb, w2_sb)
    nc.compile()
    return nc


def _prep_inputs(x, w13, w2):
    """Host-side relayout for all cores at once (f32 -> bf16)."""
    # xt: [E, H, G] = per-expert x^T, then [E, KH, 128, G]
    xt = np.ascontiguousarray(
        x.reshape(E, G, H).transpose(0, 2, 1)
    ).astype(NPBF16).reshape(E, KH, 128, G)
    # w13: [E,H,2I] -> [E, k, p, s, m, j] -> [E, m, p, k, s, j]
    a13 = np.ascontiguousarray(
        w13.reshape(E, KH, 128, 2, KI, 128).transpose(0, 4, 2, 1, 3, 5)
    ).astype(NPBF16)
    a2 = w2.reshape(E, KI, 128, H).astype(NPBF16)
    return xt, a13, a2


def _in_map_for_core(xt, a13, a2, c):
    s = slice(c * EPC, (c + 1) * EPC)
    return {"xt": xt[s], "w13": a13[s], "w2": a2[s]}


_NC_CACHE = []


def kernel(x, w13, w2, tokens_per_expert, decoding, _trace=False):
    x = np.asarray(x, dtype=np.float32)
    w13 = np.asarray(w13, dtype=np.float32)
    w2 = np.asarray(w2, dtype=np.float32)

    xt, a13, a2 = _prep_inputs(x, w13, w2)
    in_maps = [_in_map_for_core(xt, a13, a2, c) for c in range(N_CORES)]
    if not _NC_CACHE:
        _NC_CACHE.append(build_nc())
    nc = _NC_CACHE[0]
    res = run_bass_kernel_spmd(nc, in_maps, list(range(N_CORES)), trace=_trace)
    out = np.concatenate(
        [np.asarray(res.results[c]["out"]).astype(np.float32) for c in range(N_CORES)],
        axis=0,
    )
    if _trace:
        return out, res
    return out
